# revision 22
# baseline (speedup 1.0000x reference)
"""Multi-head attention (B=4, S=2048, D=1024, H=16) on 8 Trainium2 NeuronCores.

Sharding: batch (4-way data parallel) x head-group (2-way tensor parallel).
Core c handles batch c//2, heads [8*(c%2), 8*(c%2)+8).  Each core computes a
partial output [S, D] (its heads' contribution through its Wo row-slice); the
host sums the two partials per batch.

Per-core kernel (all matmuls bf16, fp32 PSUM accumulation):
  phase 0: QKV projections from pre-transposed x^T.
           Q^T/K^T stored head-pair-major: [128 = 2 heads x 64 depth, seq].
           V stored [keys, 16 chunks, 8 heads, 65]: col 64 = exp(bias[key])
           and cols 0..63 scaled by exp(bias[key]) -> exact bias support and
           the softmax denominator falls out of the E@V matmul as row 64.
  phase 1: per head-pair, per 512-wide q chunk:
           scores^T [keys,q] via row-packed K=64 matmuls (tile_position
           auto-derived from base partitions 0/64 -> both heads concurrent),
           exp on ScalarE (N=1024 per instruction), E@V with M=65,
           normalization via DVE reciprocal + K=1 broadcast matmul.
  phase 2: output projection, K=128 full-rate, accumulate head pairs.
"""

import os

os.environ.setdefault("MYCRO_LOCAL_CACHE", "1")

from contextlib import ExitStack

import numpy as np
import ml_dtypes

import concourse.bacc as bacc
import concourse.mybir as mybir
import concourse.tile as tile
from concourse.bass_utils import run_bass_kernel_spmd

BF = mybir.dt.bfloat16
F32 = mybir.dt.float32
BF_NP = ml_dtypes.bfloat16

B, S, D, H = 4, 2048, 1024, 16
DEPTH = D // H          # 64
HPC = 8                 # heads per core
FPC = HPC * DEPTH       # 512 features per core
P = 128
CH = D // P             # 8 contraction chunks for the projections
NK = S // P             # 16 key chunks
NQ = S // 512           # 4 q chunks

_NC_CACHE = {}


def _emit(ctx: ExitStack, tc, xt_d, wq_d, wk_d, wv_d, wo_d, eb_d, out_d):
    nc = tc.nc
    Exp = mybir.ActivationFunctionType.Exp

    const = ctx.enter_context(tc.tile_pool(name="const", bufs=1))
    wpool = ctx.enter_context(tc.tile_pool(name="wpool", bufs=1))
    xpool = ctx.enter_context(tc.tile_pool(name="xpool", bufs=1))
    qkpool = ctx.enter_context(tc.tile_pool(name="qkpool", bufs=1))
    vpool = ctx.enter_context(tc.tile_pool(name="vpool", bufs=1))
    epool = ctx.enter_context(tc.tile_pool(name="epool", bufs=18))
    atpool = ctx.enter_context(tc.tile_pool(name="atpool", bufs=2))
    stpool = ctx.enter_context(tc.tile_pool(name="stpool", bufs=3))
    smpool = ctx.enter_context(tc.tile_pool(name="smpool", bufs=2))
    # PSUM budget (8 banks): scores 2x[128,1024] = 4, attn 2x[65,512] = 2,
    # misc (qkv/wo/rb) 2x[<=128,512] = 2.
    ps_sc = ctx.enter_context(tc.tile_pool(name="ps_sc", bufs=2, space="PSUM"))
    ps_at = ctx.enter_context(tc.tile_pool(name="ps_at", bufs=2, space="PSUM"))
    ps_ms = ctx.enter_context(tc.tile_pool(name="ps_ms", bufs=2, space="PSUM"))

    ones = const.tile([1, DEPTH], F32)
    nc.vector.memset(ones, 1.0)
    ones128 = const.tile([P, 1], F32)
    nc.vector.memset(ones128, 1.0)
    bb_sb = const.tile([P, NK], F32)   # raw bias, chunked [key%128, chunk]
    nc.sync.dma_start(out=bb_sb, in_=eb_d)

    wq_sb = wpool.tile([P, CH, FPC], BF)
    wk_sb = wpool.tile([P, CH, FPC], BF)
    wv_sb = wpool.tile([P, CH, FPC], BF)
    wo_sb = wpool.tile([P, HPC // 2, D], BF)
    nc.sync.dma_start(out=wq_sb, in_=wq_d.rearrange("(c p) f -> p c f", p=P))
    nc.sync.dma_start(out=wk_sb, in_=wk_d.rearrange("(c p) f -> p c f", p=P))
    nc.sync.dma_start(out=wv_sb, in_=wv_d.rearrange("(c p) f -> p c f", p=P))
    nc.sync.dma_start(out=wo_sb, in_=wo_d.rearrange("(c p) f -> p c f", p=P))

    xt_sb = xpool.tile([P, CH, S], BF)
    xt_view = xt_d.rearrange("(c p) s -> p c s", p=P)
    for c in range(CH):
        nc.sync.dma_start(out=xt_sb[:, c, :], in_=xt_view[:, c, :])

    # ---- phase 0: projections, emitted lazily into the attention stream ----
    QT = qkpool.tile([P, HPC // 2, S], BF)   # [2 heads x 64 depth, pair, seq]
    KT = qkpool.tile([P, HPC // 2, S], BF)
    V = vpool.tile([P, NK, FPC], BF)         # [key%128, chunk, head*64+depth]

    def qkt_thunk(w_sb, dst, pair, sc):
        def f(w_sb=w_sb, dst=dst, pair=pair, sc=sc):
            ps = ps_ms.tile([P, 512], F32, tag="ms")
            for c in range(CH):
                nc.tensor.matmul(
                    ps,
                    lhsT=w_sb[:, c, 128 * pair:128 * (pair + 1)],
                    rhs=xt_sb[:, c, 512 * sc:512 * (sc + 1)],
                    start=(c == 0),
                    stop=(c == CH - 1),
                )
            nc.vector.tensor_copy(dst[:, pair, 512 * sc:512 * (sc + 1)], ps)
        return f

    def v_thunk(sb):
        def f(sb=sb):
            ps = ps_ms.tile([P, 512], F32, tag="ms")
            for c in range(CH):
                nc.tensor.matmul(
                    ps,
                    lhsT=xt_sb[:, c, 128 * sb:128 * (sb + 1)],
                    rhs=wv_sb[:, c, :],
                    start=(c == 0),
                    stop=(c == CH - 1),
                )
            nc.vector.tensor_copy(V[:, sb, :], ps)
        return f

    # seed: what the very first logits need, emitted eagerly
    qkt_thunk(wq_sb, QT, 0, 0)()
    for sc in range(NQ):
        qkt_thunk(wk_sb, KT, 0, sc)()
    # everything else drips into the attention loop, ordered by first use
    pending = [v_thunk(sb) for sb in range(NK)]
    for pp in range(1, HPC // 2):
        for sc in range(NQ):
            pending.append(qkt_thunk(wk_sb, KT, pp, sc))
        pending.append(qkt_thunk(wq_sb, QT, pp, 0))
    for sc in range(1, NQ):
        for pp in range(HPC // 2):
            pending.append(qkt_thunk(wq_sb, QT, pp, sc))
    pending.reverse()  # pop() from the end

    # ---- phases 1+2: attention + output projection ----
    for qc in range(NQ):
        q0 = 512 * qc
        # per-qc attnT tile: decouples Wo(qc) reads from attention(qc+1) writes
        attnT = atpool.tile([P, HPC // 2, 512], BF, tag="attnT")
        for pair in range(HPC // 2):
            hA, hB = 2 * pair, 2 * pair + 1
            atP = ps_at.tile([P, 512], F32, tag="at")

            def ev(g, atP=atP, hA=hA, hB=hB):
                # E@V col-packed: head A -> psum 0:64, head B -> 64:128
                nc.tensor.matmul(
                    atP[0:DEPTH, :],
                    lhsT=V[:, g, DEPTH * hA:DEPTH * (hA + 1)],
                    rhs=e_ts[g][:, 0:512],
                    start=(g == 0), stop=(g == NK - 1),
                    tile_position=(0, 0), skip_group_check=True,
                )
                nc.tensor.matmul(
                    atP[DEPTH:P, :],
                    lhsT=V[:, g, DEPTH * hB:DEPTH * (hB + 1)],
                    rhs=e_ts[g][:, 512:1024],
                    start=(g == 0), stop=(g == NK - 1),
                    tile_position=(0, DEPTH), skip_group_check=True,
                )

            for _ in range(6):
                if pending:
                    pending.pop()()
            # logits/exp stream with EV software-pipelined 2 slots behind so
            # neither PE (dense EV blocks) nor ACT (exp) ever starves
            e_ts = []
            for g in range(NK):
                k0 = 128 * g
                sc_t = ps_sc.tile([P, 1024], F32, tag="sc")
                # two K=64 heads row-packed (base partitions 0 / 64)
                nc.tensor.matmul(
                    sc_t[:, 0:512],
                    lhsT=KT[0:DEPTH, pair, k0:k0 + 128],
                    rhs=QT[0:DEPTH, pair, q0:q0 + 512],
                    start=True, stop=True,
                )
                nc.tensor.matmul(
                    sc_t[:, 512:1024],
                    lhsT=KT[DEPTH:P, pair, k0:k0 + 128],
                    rhs=QT[DEPTH:P, pair, q0:q0 + 512],
                    start=True, stop=True,
                )
                e_t = epool.tile([P, 1024], BF, tag="e")
                # bias folded into exp: exp(l + bias[key]); bias is per
                # partition (= key) so one [P,1] AP serves both head halves
                nc.scalar.activation(e_t, sc_t, Exp, bias=bb_sb[:, g:g + 1])
                e_ts.append(e_t)
                if g >= 2:
                    ev(g - 2)
                if pending:
                    pending.pop()()
            ev(NK - 2)
            ev(NK - 1)
            # denominators: in-place bf16 tree-sum of the 16 E tiles, then a
            # f32 ones-matmul over partitions (bf16 tree error averages out
            # 1/sqrt(128) in the partition sum -> negligible)
            for step in (1, 2, 4):
                for i in range(0, NK, 2 * step):
                    nc.vector.tensor_add(e_ts[i], e_ts[i], e_ts[i + step])
            accf = smpool.tile([P, 1024], F32, tag="accf")
            nc.vector.tensor_add(accf, e_ts[0], e_ts[8])
            dpA = ps_ms.tile([1, 512], F32, tag="ms")
            dpB = ps_ms.tile([1, 512], F32, tag="ms")
            nc.tensor.matmul(dpA, lhsT=ones128, rhs=accf[:, 0:512],
                             start=True, stop=True)
            nc.tensor.matmul(dpB, lhsT=ones128, rhs=accf[:, 512:1024],
                             start=True, stop=True)
            rA = smpool.tile([1, 512], F32, tag="recip")
            rB = smpool.tile([1, 512], F32, tag="recip")
            dA = smpool.tile([1, 512], F32, tag="den")
            dB = smpool.tile([1, 512], F32, tag="den")
            nc.vector.tensor_copy(dA, dpA)
            nc.vector.tensor_copy(dB, dpB)
            nc.vector.reciprocal_approx_fast(rA, dA)
            nc.vector.reciprocal_approx_fast(rB, dB)
            rb = ps_ms.tile([P, 512], F32, tag="ms")
            nc.tensor.matmul(rb[0:DEPTH, :], lhsT=ones, rhs=rA,
                             start=True, stop=True, tile_position=(0, 0))
            nc.tensor.matmul(rb[DEPTH:P, :], lhsT=ones, rhs=rB,
                             start=True, stop=True, tile_position=(0, DEPTH))
            # tensor_tensor may read at most one PSUM operand: stage rb in SBUF
            rs = smpool.tile([P, 512], F32, tag="rb_sb")
            nc.vector.tensor_copy(rs, rb)
            nc.vector.tensor_mul(attnT[:, pair, :], atP, rs)
        # output projection for this q chunk
        for qb in range(4):
            qq = q0 + 128 * qb
            for n in range(2):
                po = ps_ms.tile([P, 512], F32, tag="ms")
                for pair in range(HPC // 2):
                    nc.tensor.matmul(
                        po,
                        lhsT=attnT[:, pair, 128 * qb:128 * (qb + 1)],
                        rhs=wo_sb[:, pair, 512 * n:512 * (n + 1)],
                        start=(pair == 0),
                        stop=(pair == HPC // 2 - 1),
                    )
                st = stpool.tile([P, 512], F32, tag="st")
                nc.vector.tensor_copy(st, po)
                nc.sync.dma_start(
                    out=out_d[qq:qq + 128, 512 * n:512 * (n + 1)], in_=st
                )


def _build():
    nc = bacc.Bacc("TRN2", target_bir_lowering=False, debug=False)
    xt = nc.dram_tensor("xt", [D, S], BF, kind="ExternalInput").ap()
    wq = nc.dram_tensor("wq", [D, FPC], BF, kind="ExternalInput").ap()
    wk = nc.dram_tensor("wk", [D, FPC], BF, kind="ExternalInput").ap()
    wv = nc.dram_tensor("wv", [D, FPC], BF, kind="ExternalInput").ap()
    wo = nc.dram_tensor("wo", [FPC, D], BF, kind="ExternalInput").ap()
    eb = nc.dram_tensor("eb", [P, NK], F32, kind="ExternalInput").ap()
    out = nc.dram_tensor("out", [S, D], F32, kind="ExternalOutput").ap()
    with tile.TileContext(nc) as tc:
        with ExitStack() as ctx:
            _emit(ctx, tc, xt, wq, wk, wv, wo, eb, out)
    nc.compile()
    return nc


def get_nc():
    if "nc" not in _NC_CACHE:
        _NC_CACHE["nc"] = _build()
    return _NC_CACHE["nc"]


def _in_maps(x, bias, Wq, Wk, Wv, Wo):
    x = np.asarray(x, dtype=np.float32)
    bias = np.asarray(bias, dtype=np.float32)
    maps = []
    for core in range(8):
        b, grp = core // 2, core % 2
        cols = slice(FPC * grp, FPC * (grp + 1))
        xt = np.ascontiguousarray(np.asarray(x[b]).T).astype(BF_NP)
        wq = np.ascontiguousarray(np.asarray(Wq)[:, cols] * (DEPTH ** -0.5)).astype(BF_NP)
        wk = np.ascontiguousarray(np.asarray(Wk)[:, cols]).astype(BF_NP)
        wv = np.ascontiguousarray(np.asarray(Wv)[:, cols]).astype(BF_NP)
        wo = np.ascontiguousarray(np.asarray(Wo)[cols, :]).astype(BF_NP)
        eb = np.ascontiguousarray(
            bias[b, 0, 0].astype(np.float32).reshape(NK, P).T
        )  # raw bias, [128 = key%128, 16 = key chunk]
        maps.append(
            {"xt": xt, "wq": wq, "wk": wk, "wv": wv, "wo": wo, "eb": eb}
        )
    return maps


def _get_exec():
    """Cached jitted SPMD executable mirroring bass2jax.run_bass_via_pjrt,
    without donation (our kernel writes every output element) so repeated
    calls can reuse persistent device buffers for timing."""
    if "exec" in _NC_CACHE:
        return _NC_CACHE["exec"]
    import jax
    import concourse.mybir as _mybir
    from concourse.bass2jax import (
        _bass_exec_p,
        install_neuronx_cc_hook,
        partition_id_tensor,
    )
    from jax.experimental.shard_map import shard_map
    from jax.sharding import Mesh, NamedSharding, PartitionSpec

    install_neuronx_cc_hook()
    nc = get_nc()
    n_cores = 8
    part_name = nc.partition_id_tensor.name if nc.partition_id_tensor else None
    in_names, out_names, out_avals = [], [], []
    for alloc in nc.m.functions[0].allocations:
        if not isinstance(alloc, _mybir.MemoryLocationSet):
            continue
        name = alloc.memorylocations[0].name
        if alloc.kind == "ExternalInput":
            if name != part_name:
                in_names.append(name)
        elif alloc.kind == "ExternalOutput":
            out_names.append(name)
            out_avals.append(
                jax.core.ShapedArray(
                    tuple(alloc.tensor_shape), _mybir.dt.np(alloc.dtype)
                )
            )
    n_params = len(in_names)
    all_names = in_names + out_names
    if part_name is not None:
        all_names = all_names + [part_name]

    def _body(*args):
        operands = list(args)
        if part_name is not None:
            operands.append(partition_id_tensor())
        return tuple(
            _bass_exec_p.bind(
                *operands,
                out_avals=tuple(out_avals),
                in_names=tuple(all_names),
                out_names=tuple(out_names),
                lowering_input_output_aliases=(),
                sim_require_finite=True,
                sim_require_nnan=True,
                nc=nc,
            )
        )

    devices = jax.devices()[:n_cores]
    mesh = Mesh(np.asarray(devices), ("core",))
    nshard = NamedSharding(mesh, PartitionSpec("core"))
    sharded = jax.jit(
        shard_map(
            _body,
            mesh=mesh,
            in_specs=(PartitionSpec("core"),) * (n_params + len(out_names)),
            out_specs=(PartitionSpec("core"),) * len(out_names),
            check_rep=False,
        ),
        keep_unused=True,
    )
    zeros = [
        jax.device_put(
            np.zeros((n_cores * a.shape[0], *a.shape[1:]), a.dtype), nshard
        )
        for a in out_avals
    ]
    _NC_CACHE["exec"] = (sharded, in_names, out_names, out_avals, nshard, zeros)
    return _NC_CACHE["exec"]


def _execute(maps):
    import jax

    sharded, in_names, out_names, out_avals, nshard, zeros = _get_exec()
    concat_in = [
        jax.device_put(
            np.concatenate([np.asarray(m[name]) for m in maps], axis=0), nshard
        )
        for name in in_names
    ]
    outs = sharded(*concat_in, *zeros)
    return concat_in, outs, out_names, out_avals


def run(x, bias, Wq, Wk, Wv, Wo, trace=False):
    """Returns (full_output [B,S,D] f32, per-core outs)."""
    maps = _in_maps(x, bias, Wq, Wk, Wv, Wo)
    _, outs, out_names, out_avals = _execute(maps)
    per_core = np.asarray(outs[out_names.index("out")]).reshape(8, S, D)
    full = np.empty((B, S, D), dtype=np.float32)
    for b in range(B):
        full[b] = per_core[2 * b] + per_core[2 * b + 1]
    return full, per_core


def bench(x, bias, Wq, Wk, Wv, Wo, iters=20):
    """Amortized per-execution wall time (ns) over pipelined dispatches."""
    import jax
    import time

    maps = _in_maps(x, bias, Wq, Wk, Wv, Wo)
    sharded, in_names, out_names, out_avals, nshard, zeros = _get_exec()
    concat_in = [
        jax.device_put(
            np.concatenate([np.asarray(m[name]) for m in maps], axis=0), nshard
        )
        for name in in_names
    ]
    outs = sharded(*concat_in, *zeros)  # warmup / compile
    jax.block_until_ready(outs)
    t0 = time.perf_counter()
    for _ in range(iters):
        outs = sharded(*concat_in, *zeros)
    jax.block_until_ready(outs)
    dt = (time.perf_counter() - t0) / iters
    return int(dt * 1e9)


def kernel(x, bias, Wq, Wk, Wv, Wo):
    return run(x, bias, Wq, Wk, Wv, Wo)[0]


# revision 25
# speedup vs baseline: 1.1908x; 1.1908x over previous
"""Multi-head attention (B=4, S=2048, D=1024, H=16) on 8 Trainium2 NeuronCores.

Sharding: batch (4-way data parallel) x head-group (2-way tensor parallel).
Core c handles batch c//2, heads [8*(c%2), 8*(c%2)+8).  Each core computes a
partial output [S, D] (its heads' contribution through its Wo row-slice); the
host sums the two partials per batch.

Per-core kernel (all matmuls bf16, fp32 PSUM accumulation):
  phase 0: QKV projections from pre-transposed x^T.
           Q^T/K^T stored head-pair-major: [128 = 2 heads x 64 depth, seq].
           V stored [keys, 16 chunks, 8 heads, 65]: col 64 = exp(bias[key])
           and cols 0..63 scaled by exp(bias[key]) -> exact bias support and
           the softmax denominator falls out of the E@V matmul as row 64.
  phase 1: per head-pair, per 512-wide q chunk:
           scores^T [keys,q] via row-packed K=64 matmuls (tile_position
           auto-derived from base partitions 0/64 -> both heads concurrent),
           exp on ScalarE (N=1024 per instruction), E@V with M=65,
           normalization via DVE reciprocal + K=1 broadcast matmul.
  phase 2: output projection, K=128 full-rate, accumulate head pairs.
"""

import os

os.environ.setdefault("MYCRO_LOCAL_CACHE", "1")

from contextlib import ExitStack

import numpy as np
import ml_dtypes

import concourse.bacc as bacc
import concourse.mybir as mybir
import concourse.tile as tile
from concourse.bass_utils import run_bass_kernel_spmd

BF = mybir.dt.bfloat16
F32 = mybir.dt.float32
BF_NP = ml_dtypes.bfloat16

B, S, D, H = 4, 2048, 1024, 16
DEPTH = D // H          # 64
HPC = 8                 # heads per core
FPC = HPC * DEPTH       # 512 features per core
P = 128
CH = D // P             # 8 contraction chunks for the projections
NK = S // P             # 16 key chunks
NQ = S // 512           # 4 q chunks

_NC_CACHE = {}


def _emit(ctx: ExitStack, tc, xt_d, wq_d, wk_d, wv_d, wo_d, eb_d, out_d):
    nc = tc.nc
    Exp = mybir.ActivationFunctionType.Exp

    const = ctx.enter_context(tc.tile_pool(name="const", bufs=1))
    wpool = ctx.enter_context(tc.tile_pool(name="wpool", bufs=1))
    xpool = ctx.enter_context(tc.tile_pool(name="xpool", bufs=1))
    qkpool = ctx.enter_context(tc.tile_pool(name="qkpool", bufs=1))
    vpool = ctx.enter_context(tc.tile_pool(name="vpool", bufs=1))
    epool = ctx.enter_context(tc.tile_pool(name="epool", bufs=18))
    atpool = ctx.enter_context(tc.tile_pool(name="atpool", bufs=2))
    stpool = ctx.enter_context(tc.tile_pool(name="stpool", bufs=3))
    smpool = ctx.enter_context(tc.tile_pool(name="smpool", bufs=2))
    # PSUM budget (8 banks): scores 2x[128,1024] = 4, attn 2x[65,512] = 2,
    # misc (qkv/wo/rb) 2x[<=128,512] = 2.
    ps_sc = ctx.enter_context(tc.tile_pool(name="ps_sc", bufs=2, space="PSUM"))
    ps_at = ctx.enter_context(tc.tile_pool(name="ps_at", bufs=2, space="PSUM"))
    ps_ms = ctx.enter_context(tc.tile_pool(name="ps_ms", bufs=2, space="PSUM"))

    ones = const.tile([1, DEPTH], F32)
    nc.vector.memset(ones, 1.0)
    ones128 = const.tile([P, 1], F32)
    nc.vector.memset(ones128, 1.0)
    bb_sb = const.tile([P, NK], F32)   # raw bias, chunked [key%128, chunk]
    nc.sync.dma_start(out=bb_sb, in_=eb_d)

    # seed-path tensors (wq, wk, xt) first so the first logits start early
    wq_sb = wpool.tile([P, CH, FPC], BF)
    wk_sb = wpool.tile([P, CH, FPC], BF)
    wv_sb = wpool.tile([P, CH, FPC], BF)
    wo_sb = wpool.tile([P, HPC // 2, D], BF)
    nc.sync.dma_start(out=wq_sb, in_=wq_d.rearrange("(c p) f -> p c f", p=P))
    nc.sync.dma_start(out=wk_sb, in_=wk_d.rearrange("(c p) f -> p c f", p=P))
    xt_sb = xpool.tile([P, CH, S], BF)
    xt_view = xt_d.rearrange("(c p) s -> p c s", p=P)
    for c in range(CH):
        nc.sync.dma_start(out=xt_sb[:, c, :], in_=xt_view[:, c, :])
    nc.sync.dma_start(out=wv_sb, in_=wv_d.rearrange("(c p) f -> p c f", p=P))
    nc.sync.dma_start(out=wo_sb, in_=wo_d.rearrange("(c p) f -> p c f", p=P))

    # ---- phase 0: projections, emitted lazily into the attention stream ----
    QT = qkpool.tile([P, HPC // 2, S], BF)   # [2 heads x 64 depth, pair, seq]
    KT = qkpool.tile([P, HPC // 2, S], BF)
    V = vpool.tile([P, NK, FPC], BF)         # [key%128, chunk, head*64+depth]

    def qkt_thunk(w_sb, dst, pair, sc):
        def f(w_sb=w_sb, dst=dst, pair=pair, sc=sc):
            ps = ps_ms.tile([P, 512], F32, tag="ms")
            for c in range(CH):
                nc.tensor.matmul(
                    ps,
                    lhsT=w_sb[:, c, 128 * pair:128 * (pair + 1)],
                    rhs=xt_sb[:, c, 512 * sc:512 * (sc + 1)],
                    start=(c == 0),
                    stop=(c == CH - 1),
                )
            nc.vector.tensor_copy(dst[:, pair, 512 * sc:512 * (sc + 1)], ps)
        return f

    def v_thunk(sb):
        def f(sb=sb):
            ps = ps_ms.tile([P, 512], F32, tag="ms")
            for c in range(CH):
                nc.tensor.matmul(
                    ps,
                    lhsT=xt_sb[:, c, 128 * sb:128 * (sb + 1)],
                    rhs=wv_sb[:, c, :],
                    start=(c == 0),
                    stop=(c == CH - 1),
                )
            nc.vector.tensor_copy(V[:, sb, :], ps)
        return f

    # seed: only what the very first logits block needs, emitted eagerly
    qkt_thunk(wq_sb, QT, 0, 0)()
    qkt_thunk(wk_sb, KT, 0, 0)()
    # everything else drips into the attention loop, ordered by first use
    pending = [qkt_thunk(wk_sb, KT, 0, sc) for sc in range(1, NQ)]
    pending += [v_thunk(sb) for sb in range(NK)]
    for pp in range(1, HPC // 2):
        for sc in range(NQ):
            pending.append(qkt_thunk(wk_sb, KT, pp, sc))
        pending.append(qkt_thunk(wq_sb, QT, pp, 0))
    for sc in range(1, NQ):
        for pp in range(HPC // 2):
            pending.append(qkt_thunk(wq_sb, QT, pp, sc))
    pending.reverse()  # pop() from the end

    # ---- phases 1+2: attention + output projection ----
    for qc in range(NQ):
        q0 = 512 * qc
        # per-qc attnT tile: decouples Wo(qc) reads from attention(qc+1) writes
        attnT = atpool.tile([P, HPC // 2, 512], BF, tag="attnT")
        for pair in range(HPC // 2):
            hA, hB = 2 * pair, 2 * pair + 1
            atP = ps_at.tile([P, 512], F32, tag="at")

            def ev(g, atP=atP, hA=hA, hB=hB):
                # E@V col-packed: head A -> psum 0:64, head B -> 64:128
                nc.tensor.matmul(
                    atP[0:DEPTH, :],
                    lhsT=V[:, g, DEPTH * hA:DEPTH * (hA + 1)],
                    rhs=e_ts[g][:, 0:512],
                    start=(g == 0), stop=(g == NK - 1),
                    tile_position=(0, 0), skip_group_check=True,
                )
                nc.tensor.matmul(
                    atP[DEPTH:P, :],
                    lhsT=V[:, g, DEPTH * hB:DEPTH * (hB + 1)],
                    rhs=e_ts[g][:, 512:1024],
                    start=(g == 0), stop=(g == NK - 1),
                    tile_position=(0, DEPTH), skip_group_check=True,
                )

            for _ in range(6):
                if pending:
                    pending.pop()()
            # logits/exp stream with EV software-pipelined 2 slots behind so
            # neither PE (dense EV blocks) nor ACT (exp) ever starves
            e_ts = []
            for g in range(NK):
                k0 = 128 * g
                sc_t = ps_sc.tile([P, 1024], F32, tag="sc")
                # two K=64 heads row-packed (base partitions 0 / 64)
                nc.tensor.matmul(
                    sc_t[:, 0:512],
                    lhsT=KT[0:DEPTH, pair, k0:k0 + 128],
                    rhs=QT[0:DEPTH, pair, q0:q0 + 512],
                    start=True, stop=True,
                )
                nc.tensor.matmul(
                    sc_t[:, 512:1024],
                    lhsT=KT[DEPTH:P, pair, k0:k0 + 128],
                    rhs=QT[DEPTH:P, pair, q0:q0 + 512],
                    start=True, stop=True,
                )
                e_t = epool.tile([P, 1024], BF, tag="e")
                # bias folded into exp: exp(l + bias[key]); bias is per
                # partition (= key) so one [P,1] AP serves both head halves
                nc.scalar.activation(e_t, sc_t, Exp, bias=bb_sb[:, g:g + 1])
                e_ts.append(e_t)
                if g >= 2:
                    ev(g - 2)
                for _ in range(2 if pair == 0 and qc == 0 else 1):
                    if pending:
                        pending.pop()()
            ev(NK - 2)
            ev(NK - 1)
            # denominators: in-place bf16 tree-sum of the 16 E tiles, then a
            # f32 ones-matmul over partitions (bf16 tree error averages out
            # 1/sqrt(128) in the partition sum -> negligible)
            for step in (1, 2, 4):
                for i in range(0, NK, 2 * step):
                    nc.vector.tensor_add(e_ts[i], e_ts[i], e_ts[i + step])
            accf = smpool.tile([P, 1024], F32, tag="accf")
            nc.vector.tensor_add(accf, e_ts[0], e_ts[8])
            dpA = ps_ms.tile([1, 512], F32, tag="ms")
            dpB = ps_ms.tile([1, 512], F32, tag="ms")
            nc.tensor.matmul(dpA, lhsT=ones128, rhs=accf[:, 0:512],
                             start=True, stop=True)
            nc.tensor.matmul(dpB, lhsT=ones128, rhs=accf[:, 512:1024],
                             start=True, stop=True)
            rA = smpool.tile([1, 512], F32, tag="recip")
            rB = smpool.tile([1, 512], F32, tag="recip")
            dA = smpool.tile([1, 512], F32, tag="den")
            dB = smpool.tile([1, 512], F32, tag="den")
            nc.vector.tensor_copy(dA, dpA)
            nc.vector.tensor_copy(dB, dpB)
            nc.vector.reciprocal_approx_fast(rA, dA)
            nc.vector.reciprocal_approx_fast(rB, dB)
            rb = ps_ms.tile([P, 512], F32, tag="ms")
            nc.tensor.matmul(rb[0:DEPTH, :], lhsT=ones, rhs=rA,
                             start=True, stop=True, tile_position=(0, 0))
            nc.tensor.matmul(rb[DEPTH:P, :], lhsT=ones, rhs=rB,
                             start=True, stop=True, tile_position=(0, DEPTH))
            # tensor_tensor may read at most one PSUM operand: stage rb in SBUF
            rs = smpool.tile([P, 512], F32, tag="rb_sb")
            nc.vector.tensor_copy(rs, rb)
            nc.vector.tensor_mul(attnT[:, pair, :], atP, rs)
        # output projection for this q chunk
        for qb in range(4):
            qq = q0 + 128 * qb
            for n in range(2):
                po = ps_ms.tile([P, 512], F32, tag="ms")
                for pair in range(HPC // 2):
                    nc.tensor.matmul(
                        po,
                        lhsT=attnT[:, pair, 128 * qb:128 * (qb + 1)],
                        rhs=wo_sb[:, pair, 512 * n:512 * (n + 1)],
                        start=(pair == 0),
                        stop=(pair == HPC // 2 - 1),
                    )
                st = stpool.tile([P, 512], F32, tag="st")
                nc.vector.tensor_copy(st, po)
                nc.sync.dma_start(
                    out=out_d[qq:qq + 128, 512 * n:512 * (n + 1)], in_=st
                )


def _build():
    nc = bacc.Bacc("TRN2", target_bir_lowering=False, debug=False)
    xt = nc.dram_tensor("xt", [D, S], BF, kind="ExternalInput").ap()
    wq = nc.dram_tensor("wq", [D, FPC], BF, kind="ExternalInput").ap()
    wk = nc.dram_tensor("wk", [D, FPC], BF, kind="ExternalInput").ap()
    wv = nc.dram_tensor("wv", [D, FPC], BF, kind="ExternalInput").ap()
    wo = nc.dram_tensor("wo", [FPC, D], BF, kind="ExternalInput").ap()
    eb = nc.dram_tensor("eb", [P, NK], F32, kind="ExternalInput").ap()
    out = nc.dram_tensor("out", [S, D], F32, kind="ExternalOutput").ap()
    with tile.TileContext(nc) as tc:
        with ExitStack() as ctx:
            _emit(ctx, tc, xt, wq, wk, wv, wo, eb, out)
    nc.compile()
    return nc


def get_nc():
    if "nc" not in _NC_CACHE:
        _NC_CACHE["nc"] = _build()
    return _NC_CACHE["nc"]


def _in_maps(x, bias, Wq, Wk, Wv, Wo):
    x = np.asarray(x, dtype=np.float32)
    bias = np.asarray(bias, dtype=np.float32)
    maps = []
    for core in range(8):
        b, grp = core // 2, core % 2
        cols = slice(FPC * grp, FPC * (grp + 1))
        xt = np.ascontiguousarray(np.asarray(x[b]).T).astype(BF_NP)
        wq = np.ascontiguousarray(np.asarray(Wq)[:, cols] * (DEPTH ** -0.5)).astype(BF_NP)
        wk = np.ascontiguousarray(np.asarray(Wk)[:, cols]).astype(BF_NP)
        wv = np.ascontiguousarray(np.asarray(Wv)[:, cols]).astype(BF_NP)
        wo = np.ascontiguousarray(np.asarray(Wo)[cols, :]).astype(BF_NP)
        eb = np.ascontiguousarray(
            bias[b, 0, 0].astype(np.float32).reshape(NK, P).T
        )  # raw bias, [128 = key%128, 16 = key chunk]
        maps.append(
            {"xt": xt, "wq": wq, "wk": wk, "wv": wv, "wo": wo, "eb": eb}
        )
    return maps


def _get_exec():
    """Cached jitted SPMD executable mirroring bass2jax.run_bass_via_pjrt,
    without donation (our kernel writes every output element) so repeated
    calls can reuse persistent device buffers for timing."""
    if "exec" in _NC_CACHE:
        return _NC_CACHE["exec"]
    import jax
    import concourse.mybir as _mybir
    from concourse.bass2jax import (
        _bass_exec_p,
        install_neuronx_cc_hook,
        partition_id_tensor,
    )
    from jax.experimental.shard_map import shard_map
    from jax.sharding import Mesh, NamedSharding, PartitionSpec

    install_neuronx_cc_hook()
    nc = get_nc()
    n_cores = 8
    part_name = nc.partition_id_tensor.name if nc.partition_id_tensor else None
    in_names, out_names, out_avals = [], [], []
    for alloc in nc.m.functions[0].allocations:
        if not isinstance(alloc, _mybir.MemoryLocationSet):
            continue
        name = alloc.memorylocations[0].name
        if alloc.kind == "ExternalInput":
            if name != part_name:
                in_names.append(name)
        elif alloc.kind == "ExternalOutput":
            out_names.append(name)
            out_avals.append(
                jax.core.ShapedArray(
                    tuple(alloc.tensor_shape), _mybir.dt.np(alloc.dtype)
                )
            )
    n_params = len(in_names)
    all_names = in_names + out_names
    if part_name is not None:
        all_names = all_names + [part_name]

    def _body(*args):
        operands = list(args)
        if part_name is not None:
            operands.append(partition_id_tensor())
        return tuple(
            _bass_exec_p.bind(
                *operands,
                out_avals=tuple(out_avals),
                in_names=tuple(all_names),
                out_names=tuple(out_names),
                lowering_input_output_aliases=(),
                sim_require_finite=True,
                sim_require_nnan=True,
                nc=nc,
            )
        )

    devices = jax.devices()[:n_cores]
    mesh = Mesh(np.asarray(devices), ("core",))
    nshard = NamedSharding(mesh, PartitionSpec("core"))
    sharded = jax.jit(
        shard_map(
            _body,
            mesh=mesh,
            in_specs=(PartitionSpec("core"),) * (n_params + len(out_names)),
            out_specs=(PartitionSpec("core"),) * len(out_names),
            check_rep=False,
        ),
        keep_unused=True,
    )
    zeros = [
        jax.device_put(
            np.zeros((n_cores * a.shape[0], *a.shape[1:]), a.dtype), nshard
        )
        for a in out_avals
    ]
    _NC_CACHE["exec"] = (sharded, in_names, out_names, out_avals, nshard, zeros)
    return _NC_CACHE["exec"]


def _execute(maps):
    import jax

    sharded, in_names, out_names, out_avals, nshard, zeros = _get_exec()
    concat_in = [
        jax.device_put(
            np.concatenate([np.asarray(m[name]) for m in maps], axis=0), nshard
        )
        for name in in_names
    ]
    outs = sharded(*concat_in, *zeros)
    return concat_in, outs, out_names, out_avals


def run(x, bias, Wq, Wk, Wv, Wo, trace=False):
    """Returns (full_output [B,S,D] f32, per-core outs)."""
    maps = _in_maps(x, bias, Wq, Wk, Wv, Wo)
    _, outs, out_names, out_avals = _execute(maps)
    per_core = np.asarray(outs[out_names.index("out")]).reshape(8, S, D)
    full = np.empty((B, S, D), dtype=np.float32)
    for b in range(B):
        full[b] = per_core[2 * b] + per_core[2 * b + 1]
    return full, per_core


def bench(x, bias, Wq, Wk, Wv, Wo, iters=20):
    """Amortized per-execution wall time (ns) over pipelined dispatches."""
    import jax
    import time

    maps = _in_maps(x, bias, Wq, Wk, Wv, Wo)
    sharded, in_names, out_names, out_avals, nshard, zeros = _get_exec()
    concat_in = [
        jax.device_put(
            np.concatenate([np.asarray(m[name]) for m in maps], axis=0), nshard
        )
        for name in in_names
    ]
    outs = sharded(*concat_in, *zeros)  # warmup / compile
    jax.block_until_ready(outs)
    t0 = time.perf_counter()
    for _ in range(iters):
        outs = sharded(*concat_in, *zeros)
    jax.block_until_ready(outs)
    dt = (time.perf_counter() - t0) / iters
    return int(dt * 1e9)


def kernel(x, bias, Wq, Wk, Wv, Wo):
    return run(x, bias, Wq, Wk, Wv, Wo)[0]
